# revision 21
# baseline (speedup 1.0000x reference)
"""Trainium2 Bass kernel for fused attention (QKV proj + RoPE + SDPA + o_proj).

Sharding: Megatron-style tensor parallel over heads (4 heads/core x 8 cores)
for QKV+SDPA, then per-batch AllToAll quarters switch to token parallelism for
o_proj, so each core emits a disjoint (transposed) slice of the final output.

Key perf structure vs the v1 kernel:
 - RoPE rotate-half runs as a PE matmul against a constant permutation matrix
   (no partition-swap DMAs).
 - Softmax uses a ones-column in V for the denominator, reciprocal_approx_fast
   on DVE, and a gpsimd partition_broadcast (no DRAM round trip).
 - The AllToAll is split into 4 per-batch quarters issued as soon as each
   batch's SDPA finishes, overlapping compute; o_proj weights stream while the
   last batch's SDPA still runs.
 - SDPA(b) and QKV(b+1) are emission-interleaved so the tensor engine stays
   dense (avoids pstate downclock) while ACT does the exp work.
 - o_proj runs transposed (w_o stationary) so w_o is read from HBM once.
"""
import sys

import numpy as np

try:
    import concourse.bass as bass
except ImportError:  # fresh grading env: make the toolchain importable
    for p in (
        "/root/.axon_site",
        "/root/.axon_site/_ro/trn_rl_repo",
        "/root/.axon_site/_ro/pypackages",
        "/opt/trn_rl_repo",
        "/opt/pypackages",
    ):
        if p not in sys.path:
            sys.path.append(p)
    import concourse.bass as bass

import concourse.bacc as bacc
import concourse.mybir as mybir
import concourse.tile as tile
from concourse.bass_utils import run_bass_kernel_spmd

F32 = mybir.dt.float32
F32R = mybir.dt.float32r
BF16 = mybir.dt.bfloat16
MULT = mybir.AluOpType.mult
ADD = mybir.AluOpType.add
EXP = mybir.ActivationFunctionType.Exp
LN = mybir.ActivationFunctionType.Ln

# problem dims (hardcoded for nn_Attention_42846593744909)
B, S, D = 4, 1024, 2048
H, HD = 32, 64
N_CORES = 8
H_LOC = H // N_CORES  # heads per core


def build_attention(b=B, s=S, d=D, h_loc=H_LOC, hd=HD, n_cores=N_CORES, debug=False):
    """Build the per-core SPMD Bass program. Returns finalized nc."""
    P = 128
    T = b * s                 # total tokens (4096)
    TS = T // n_cores         # output token slice per core (512)
    DCH = d // P              # contraction chunks for D (16)
    QBLK = h_loc * hd         # 256: q (or k, or v) width per core
    TCH = 256                 # qkv token chunk
    NTC = s // TCH            # 4
    QT = 512                  # query-tile width in SDPA
    NQT = s // QT             # 2
    KTC = s // P              # key chunks of 128 (8)
    ECH = n_cores * QBLK // P  # o_proj contraction chunks (16)
    SH = s // n_cores         # shard tokens per core per batch-quarter (128)
    DC = d // P               # o_proj dout chunks (16)
    EVA = h_loc * (hd + 1)    # v + ones columns (260)

    nc = bacc.Bacc()
    hidden_t = nc.dram_tensor("hidden_t", [d, T], F32R, kind="ExternalInput")
    w_qk_t = nc.dram_tensor("w_qk_t", [d, 2 * QBLK], F32R, kind="ExternalInput")
    w_v_t = nc.dram_tensor("w_v_t", [d, QBLK], F32R, kind="ExternalInput")
    wo2 = nc.dram_tensor("wo2", [DC, P, ECH * P], BF16, kind="ExternalInput")
    cos2 = nc.dram_tensor("cos2", [P, s], F32, kind="ExternalInput")
    sinrot2 = nc.dram_tensor("sinrot2", [P, s], F32, kind="ExternalInput")
    rotm_d = nc.dram_tensor("rotm", [P, P], F32R, kind="ExternalInput")
    ones_d = nc.dram_tensor("onesc", [P, 64], F32R, kind="ExternalInput")
    out_t = nc.dram_tensor("out_t", [d, TS], F32, kind="ExternalOutput")
    if debug:
        dbg_qk = nc.dram_tensor("dbg_qk", [P, 6, s], F32, kind="ExternalOutput")
        dbg_v = nc.dram_tensor("dbg_v", [P, s // P, EVA], BF16, kind="ExternalOutput")
        dbg_ao = nc.dram_tensor("dbg_ao", [b, hd, h_loc, s], BF16, kind="ExternalOutput")
        dbg_asl = nc.dram_tensor("dbg_asl", [P, ECH, TS], BF16, kind="ExternalOutput")
        dbg_sm = nc.dram_tensor("dbg_sm", [2 + 64, 512], F32, kind="ExternalOutput")

    hid_v = hidden_t[:].rearrange("(c p) t -> p c t", p=P)
    wqk_v = w_qk_t[:].rearrange("(c p) e -> p c e", p=P)
    wv_v = w_v_t[:].rearrange("(c p) e -> p c e", p=P)

    with tile.TileContext(nc) as tc:
        with (
            tc.tile_pool(name="dramp", bufs=1, space="DRAM") as dramp,
            tc.tile_pool(name="tabs", bufs=1) as tabs,
            tc.tile_pool(name="qkp", bufs=2) as qkp,
            tc.tile_pool(name="vp", bufs=2) as vp,
            tc.tile_pool(name="ep", bufs=3) as ep,
            tc.tile_pool(name="ropep", bufs=2) as ropep,
            tc.tile_pool(name="dnp", bufs=2) as dnp,
            tc.tile_pool(name="dnrp", bufs=2) as dnrp,
            tc.tile_pool(name="aop", bufs=1) as aop,
            tc.tile_pool(name="psQ", bufs=3, space="PSUM") as psQ,
            tc.tile_pool(name="psS", bufs=3, space="PSUM") as psS,
            tc.tile_pool(name="psO", bufs=2, space="PSUM") as psO,
        ):
            cc_in_h = [dramp.tile([n_cores, 2 * P, 2 * SH], BF16, name=f"cc_in_{q}")
                       for q in range(b // 2)]
            cc_out_h = [dramp.tile([n_cores, 2 * P, 2 * SH], BF16, name=f"cc_out_{q}")
                        for q in range(b // 2)]

            cos_sb = tabs.tile([P, s], F32)
            sin_sb = tabs.tile([P, s], F32)
            rotm_sb = tabs.tile([P, P], F32R)
            ones_sb = tabs.tile([P, 64], F32R)
            nc.sync.dma_start(cos_sb[:], cos2[:])
            nc.sync.dma_start(sin_sb[:], sinrot2[:])
            nc.sync.dma_start(rotm_sb[:], rotm_d[:])
            nc.sync.dma_start(ones_sb[:], ones_d[:])

            qk_tiles = {}
            v_tiles = {}
            ao_tiles = {}

            def qkv_units(bi):
                """Generator: emit QKV proj + RoPE for batch bi in small PE
                units so the driver can weave them between SDPA steps."""
                qk_tiles[bi] = qkp.tile([P, 6, s], F32R, tag="qk", name="qk_t")
                v_tiles[bi] = vp.tile([P, KTC, EVA], BF16, tag="v", name="v_t")
                for h in range(h_loc):
                    nc.scalar.activation(
                        v_tiles[bi][:, :, h * (hd + 1) + hd:h * (hd + 1) + hd + 1],
                        wv_sb[:, 0:KTC, 0:1],
                        mybir.ActivationFunctionType.Identity,
                        bias=1.0, scale=0.0)
                qk_t = qk_tiles[bi]
                v_t = v_tiles[bi]
                # zero the unused half of each per-head q chunk so the scores
                # matmul can run full-128-contraction (stale lhsT rows hit 0s)
                for h in range(h_loc):
                    z0 = (1 - h % 2) * 64
                    nc.gpsimd.memset(
                        qk_t[z0:z0 + 64, h:h + 1, :].bitcast(F32), 0.0)
                yield
                for tci in range(NTC):
                    s0 = tci * TCH
                    t0 = bi * s + s0
                    hid_sb = hidp.tile([P, DCH, TCH], F32R, tag="hid")
                    nc.sync.dma_start(hid_sb[:], hid_v[:, :, t0:t0 + TCH])
                    yield
                    for tsub in range(TCH // P):
                        kc = tci * (TCH // P) + tsub
                        psv = psQ.tile([P, QT], F32, tag="psq", name="psv")[:, 0:QBLK]
                        for dd in range(DCH):
                            nc.tensor.matmul(
                                psv[:], lhsT=hid_sb[:, dd, tsub * P:(tsub + 1) * P],
                                rhs=wv_sb[:, dd, :],
                                start=(dd == 0), stop=(dd == DCH - 1),
                            )
                            if dd % 4 == 3:
                                yield
                        nc.vector.tensor_copy(
                            v_t[:, kc:kc + 1, :].rearrange(
                                "p o (h e) -> p (o h) e", h=h_loc)[:, :, 0:hd],
                            psv.rearrange("p (h e) -> p h e", e=hd),
                        )
                    for ec in range(4):
                        ps = psQ.tile([P, QT], F32, tag="psq", name="psqk")[:, 0:TCH]
                        for dd in range(DCH):
                            nc.tensor.matmul(
                                ps[:], lhsT=wqk_sb[:, dd, ec * P:(ec + 1) * P],
                                rhs=hid_sb[:, dd, :],
                                start=(dd == 0), stop=(dd == DCH - 1),
                            )
                            if dd % 4 == 3:
                                yield
                        raw = ropep.tile([P, TCH], F32R, tag="raw")
                        nc.scalar.copy(raw[:], ps[:])
                        psr = psQ.tile([P, QT], F32, tag="psq", name="psr")[:, 0:TCH]
                        nc.tensor.matmul(psr[:], lhsT=rotm_sb[:], rhs=raw[:],
                                         start=True, stop=True)
                        cp = ropep.tile([P, TCH], F32, tag="cp")
                        nc.vector.tensor_tensor(cp[:], ps[:], cos_sb[:, s0:s0 + TCH], MULT)
                        sw = ropep.tile([P, TCH], F32, tag="sw")
                        nc.vector.tensor_tensor(sw[:], psr[:], sin_sb[:, s0:s0 + TCH], MULT)
                        if ec < 2:
                            # q: head 2*ec in rows 0:64, head 2*ec+1 in 64:128
                            nc.vector.tensor_tensor(
                                qk_t[0:64, 2 * ec, s0:s0 + TCH],
                                cp[0:64, :], sw[0:64, :], ADD)
                            nc.vector.tensor_tensor(
                                qk_t[64:128, 2 * ec + 1, s0:s0 + TCH],
                                cp[64:128, :], sw[64:128, :], ADD)
                        else:
                            nc.vector.tensor_tensor(
                                qk_t[:, 2 + ec, s0:s0 + TCH], cp[:], sw[:], ADD)
                        yield

            def pull(gen, n):
                if gen is None:
                    return
                for _ in range(n):
                    try:
                        next(gen)
                    except StopIteration:
                        return

            def make_normalize(bi, h, qt, pso):
                """Deferred normalize: emitted a few SDPA steps later so the
                PE queue never head-of-line blocks on the ACT Ln/Exp chain."""
                ao_all = ao_tiles[bi]
                q0 = qt * QT
                # 1/denom = exp(-ln(denom)) on ACT (Ln and Exp share a table)
                dnln = dnp.tile([hd + 1, QT], F32, tag="dnln", name="dnln")
                nc.scalar.activation(dnln[hd:hd + 1, :], pso[hd:hd + 1, :], LN)
                dninv = dnp.tile([hd + 1, QT], F32R, tag="dninv", name="dninv")
                nc.scalar.activation(dninv[hd:hd + 1, :], dnln[hd:hd + 1, :],
                                     EXP, scale=-1.0)

                def fin():
                    rep = psS.tile([P, QT], F32, tag="pss", name="rep")[0:hd, :]
                    nc.tensor.matmul(rep[:], lhsT=ones_sb[hd:hd + 1, 0:hd],
                                     rhs=dninv[hd:hd + 1, :], start=True, stop=True)
                    dnrep = dnrp.tile([hd, QT], F32, tag="dnrep", name="dnrep")
                    nc.vector.tensor_copy(dnrep[:], rep[:])
                    nc.vector.tensor_tensor(
                        ao_all[:, h, q0:q0 + QT], pso[0:hd, :], dnrep[:], MULT)
                return fin

            def emit_sdpa_batch(bi, gen, kt_pull_mod=1):
                """All 8 SDPA groups of batch bi, weaving in the next batch's
                QKV units (gen) to keep the PE dense and warm."""
                ao_tiles[bi] = aop.tile([hd, h_loc, s], BF16, tag="ao", name="ao_all")
                qk_t = qk_tiles[bi]
                v_t = v_tiles[bi]
                pending = None
                for h in range(h_loc):
                    for qt in range(NQT):
                        kchunk = 4 + h // 2
                        q0 = qt * QT
                        pso = psO.tile([hd + 1, QT], F32, tag="pso", name="pso")
                        prev_e = None
                        for kt in range(KTC):
                            ss = psS.tile([P, QT], F32, tag="pss", name="ss")
                            nc.tensor.matmul(
                                ss[:],
                                lhsT=qk_t[:, kchunk, kt * P:(kt + 1) * P],
                                rhs=qk_t[:, h, q0:q0 + QT],
                                start=True, stop=True,
                            )
                            e_t = ep.tile([P, QT], BF16, tag="e", name="e_t")
                            nc.scalar.activation(e_t[:], ss[:], EXP)
                            if prev_e is not None:
                                nc.tensor.matmul(
                                    pso[:],
                                    lhsT=v_t[:, kt - 1, h * (hd + 1):(h + 1) * (hd + 1)],
                                    rhs=prev_e[:],
                                    start=(kt == 1), stop=False,
                                )
                            prev_e = e_t
                            if kt % kt_pull_mod == 0:
                                pull(gen, 1)
                            if kt == 2 and pending is not None:
                                pending()
                                pending = None
                        nc.tensor.matmul(
                            pso[:],
                            lhsT=v_t[:, KTC - 1, h * (hd + 1):(h + 1) * (hd + 1)],
                            rhs=prev_e[:],
                            start=False, stop=True,
                        )
                        pending = make_normalize(bi, h, qt, pso)
                        pull(gen, 2)
                pull(gen, 4)
                pending()
                if kt_pull_mod == 1:
                    pull(gen, 1000)  # drain leftovers before staging
                # stage this batch's attention outputs for the AllToAll
                ao_all = ao_tiles[bi]
                h2 = bi // 2
                for jj in range(4):
                    j = (bi % 2) * 4 + jj
                    dst = cc_in_h[h2][j].rearrange("(h p) t -> p h t", p=hd)
                    nc.scalar.dma_start(dst, ao_all[:, :, jj * 2 * SH:(jj + 1) * 2 * SH])
                if debug:
                    nc.sync.dma_start(dbg_ao[bi], ao_all[:])
                    if bi == 0:
                        nc.sync.dma_start(dbg_qk[:], qk_tiles[0][:].bitcast(F32))
                        nc.sync.dma_start(dbg_v[:], v_tiles[0][:])
                if bi % 2 == 1:
                    nc.gpsimd.collective_compute(
                        "AllToAll",
                        mybir.AluOpType.bypass,
                        replica_groups=[list(range(n_cores))],
                        ins=[cc_in_h[h2].opt()],
                        outs=[cc_out_h[h2].opt()],
                    )
                pull(gen, 1000)  # leftover PE work fills the collective wait

            with (
                tc.tile_pool(name="wqkp", bufs=1) as wqkp,
                tc.tile_pool(name="hidp", bufs=2) as hidp,
            ):
                wqk_sb = wqkp.tile([P, DCH, 2 * QBLK], F32R)
                wv_sb = wqkp.tile([P, DCH, QBLK], F32R)
                for dd4 in range(0, DCH, DCH // 2):
                    nc.sync.dma_start(wv_sb[:, dd4:dd4 + DCH // 2],
                                      wv_v[:, dd4:dd4 + DCH // 2])
                for dd4 in range(0, DCH, DCH // 4):
                    nc.sync.dma_start(wqk_sb[:, dd4:dd4 + DCH // 4],
                                      wqk_v[:, dd4:dd4 + DCH // 4])

                g0 = qkv_units(0)
                pull(g0, 1000)
                for bi in range(b - 1):
                    emit_sdpa_batch(bi, qkv_units(bi + 1))

            # wqk/wv/hid SBUF released here -> o_proj pools alias that space
            with (
                tc.tile_pool(name="aslp", bufs=1) as aslp,
                tc.tile_pool(name="wop", bufs=4) as wop,
                tc.tile_pool(name="outp", bufs=3) as outp,
            ):
                asl = aslp.tile([P, ECH, TS], BF16)
                cc_v = cc_out_h[0][:].rearrange("j (ci p) t -> p (j ci) t", p=P)
                nc.sync.dma_start(asl[:, :, 0:2 * SH], cc_v)

                def oproj_units(h2):
                    """Transposed o_proj for token half h2:
                    out_t[:, h2*256:(h2+1)*256] = w_o @ asl[:, :, half]."""
                    c0 = h2 * 2 * SH
                    for dc in range(DC):
                        wo_sb = wop.tile([P, ECH * P], BF16, tag="wo", name="wo_sb")
                        nc.sync.dma_start(wo_sb[:], wo2[dc])
                        yield
                        pso = psQ.tile([P, QT], F32, tag="psq",
                                       name="psop")[:, 0:2 * SH]
                        for e in range(ECH):
                            nc.tensor.matmul(
                                pso[:], lhsT=wo_sb[:, e * P:(e + 1) * P],
                                rhs=asl[:, e, c0:c0 + 2 * SH],
                                start=(e == 0), stop=(e == ECH - 1),
                            )
                            if e % 8 == 7:
                                yield
                        ob = outp.tile([P, 2 * SH], F32, tag="ob", name="ob")
                        nc.vector.tensor_copy(ob[:], pso[:])
                        nc.scalar.dma_start(
                            out_t[dc * P:(dc + 1) * P, c0:c0 + 2 * SH], ob[:])
                        yield

                emit_sdpa_batch(b - 1, oproj_units(0), kt_pull_mod=2)
                cc_v = cc_out_h[1][:].rearrange("j (ci p) t -> p (j ci) t", p=P)
                nc.sync.dma_start(asl[:, :, 2 * SH:4 * SH], cc_v)

                if debug:
                    nc.sync.dma_start(dbg_asl[:], asl[:])
                for _ in oproj_units(1):
                    pass
    nc.finalize()
    return nc


def prep_inputs(cos, sin, hidden_states, w_qkv, w_o,
                b=B, s=S, d=D, h_loc=H_LOC, hd=HD, n_cores=N_CORES):
    """Host-side sharding/layout: returns per-core input maps."""
    cos = np.asarray(cos, dtype=np.float32)
    sin = np.asarray(sin, dtype=np.float32)
    hidden_states = np.asarray(hidden_states, dtype=np.float32)
    w_qkv = np.asarray(w_qkv, dtype=np.float32)
    w_o = np.asarray(w_o, dtype=np.float32)

    T = b * s
    P = 128
    QBLK = h_loc * hd
    HHD = n_cores * QBLK  # total H*HD

    hidden_t = np.ascontiguousarray(hidden_states.reshape(T, d).T)

    # wo2[dc, p, ec*128+j] = w_o[dc*128+j, ec*128+p]
    import ml_dtypes
    wo2 = np.ascontiguousarray(
        w_o.reshape(d // P, P, HHD // P, P).transpose(0, 3, 2, 1)
        .reshape(d // P, P, HHD)).astype(ml_dtypes.bfloat16)

    cos_t = cos.T  # [hd, s]
    sin_t = sin.T
    cos2 = np.ascontiguousarray(np.tile(cos_t, (P // hd, 1)))
    srt = sin_t.copy()
    srt[0:hd // 2] = -sin_t[0:hd // 2]
    sinrot2 = np.ascontiguousarray(np.tile(srt, (P // hd, 1)))

    onesc = np.ones((P, 64), dtype=np.float32)

    # rotate-half permutation (sign lives in sinrot2)
    rotm = np.zeros((P, P), dtype=np.float32)
    for m in range(P):
        pp = m + hd // 2 if (m % hd) < hd // 2 else m - hd // 2
        rotm[pp, m] = 1.0

    maps = []
    for c in range(n_cores):
        wq = w_qkv[c * QBLK:(c + 1) * QBLK] * 0.125
        wk = w_qkv[HHD + c * QBLK:HHD + (c + 1) * QBLK]
        wv = w_qkv[2 * HHD + c * QBLK:2 * HHD + (c + 1) * QBLK]
        w_qk_t = np.ascontiguousarray(np.concatenate([wq, wk], axis=0).T)
        w_v_t = np.ascontiguousarray(wv.T)
        maps.append({
            "hidden_t": hidden_t,
            "w_qk_t": w_qk_t,
            "w_v_t": w_v_t,
            "wo2": wo2,
            "cos2": cos2,
            "sinrot2": sinrot2,
            "rotm": rotm,
            "onesc": onesc,
        })
    return maps


_NC_CACHE = {}


def run(inputs, trace=False, dims=None, debug=False):
    """Run the distributed kernel. Returns (full_output, BassKernelResults)."""
    dims = dims or dict(b=B, s=S, d=D, h_loc=H_LOC, hd=HD, n_cores=N_CORES)
    key = tuple(sorted(dims.items())) + (debug,)
    if key not in _NC_CACHE:
        _NC_CACHE[key] = build_attention(debug=debug, **dims)
    nc = _NC_CACHE[key]
    maps = prep_inputs(inputs["cos"], inputs["sin"], inputs["hidden_states"],
                       inputs["w_qkv"], inputs["w_o"], **dims)
    res = run_bass_kernel_spmd(nc, maps, list(range(dims["n_cores"])), trace=trace)
    n_cores = dims["n_cores"]
    b, s, d = dims["b"], dims["s"], dims["d"]
    SH2 = 2 * s // n_cores                       # 256 tokens per core per half
    out = np.empty((2, n_cores, SH2, d), dtype=np.float32)
    for c in range(n_cores):
        ot = res.results[c]["out_t"]            # [d, 2*SH2]
        out[:, c] = ot.reshape(d, 2, SH2).transpose(1, 2, 0)
    return out.reshape(b, s, d), res


def kernel(**inputs) -> np.ndarray:
    out, _ = run(inputs)
    return out


# revision 22
# speedup vs baseline: 1.1390x; 1.1390x over previous
"""Trainium2 Bass kernel for fused attention (QKV proj + RoPE + SDPA + o_proj).

Sharding: Megatron-style tensor parallel over heads (4 heads/core x 8 cores)
for QKV+SDPA, then per-batch AllToAll quarters switch to token parallelism for
o_proj, so each core emits a disjoint (transposed) slice of the final output.

Key perf structure vs the v1 kernel:
 - RoPE rotate-half runs as a PE matmul against a constant permutation matrix
   (no partition-swap DMAs).
 - Softmax uses a ones-column in V for the denominator, reciprocal_approx_fast
   on DVE, and a gpsimd partition_broadcast (no DRAM round trip).
 - The AllToAll is split into 4 per-batch quarters issued as soon as each
   batch's SDPA finishes, overlapping compute; o_proj weights stream while the
   last batch's SDPA still runs.
 - SDPA(b) and QKV(b+1) are emission-interleaved so the tensor engine stays
   dense (avoids pstate downclock) while ACT does the exp work.
 - o_proj runs transposed (w_o stationary) so w_o is read from HBM once.
"""
import sys

import numpy as np

try:
    import concourse.bass as bass
except ImportError:  # fresh grading env: make the toolchain importable
    for p in (
        "/root/.axon_site",
        "/root/.axon_site/_ro/trn_rl_repo",
        "/root/.axon_site/_ro/pypackages",
        "/opt/trn_rl_repo",
        "/opt/pypackages",
    ):
        if p not in sys.path:
            sys.path.append(p)
    import concourse.bass as bass

import concourse.bacc as bacc
import concourse.mybir as mybir
import concourse.tile as tile
from concourse.bass_utils import run_bass_kernel_spmd

F32 = mybir.dt.float32
F32R = mybir.dt.float32r
BF16 = mybir.dt.bfloat16
MULT = mybir.AluOpType.mult
ADD = mybir.AluOpType.add
EXP = mybir.ActivationFunctionType.Exp
LN = mybir.ActivationFunctionType.Ln

# problem dims (hardcoded for nn_Attention_42846593744909)
B, S, D = 4, 1024, 2048
H, HD = 32, 64
N_CORES = 8
H_LOC = H // N_CORES  # heads per core


def build_attention(b=B, s=S, d=D, h_loc=H_LOC, hd=HD, n_cores=N_CORES, debug=False):
    """Build the per-core SPMD Bass program. Returns finalized nc."""
    P = 128
    T = b * s                 # total tokens (4096)
    TS = T // n_cores         # output token slice per core (512)
    DCH = d // P              # contraction chunks for D (16)
    QBLK = h_loc * hd         # 256: q (or k, or v) width per core
    TCH = 256                 # qkv token chunk
    NTC = s // TCH            # 4
    QT = 512                  # query-tile width in SDPA
    NQT = s // QT             # 2
    KTC = s // P              # key chunks of 128 (8)
    ECH = n_cores * QBLK // P  # o_proj contraction chunks (16)
    SH = s // n_cores         # shard tokens per core per batch-quarter (128)
    DC = d // P               # o_proj dout chunks (16)
    EVA = h_loc * (hd + 1)    # v + ones columns (260)

    nc = bacc.Bacc()
    hidden_t = nc.dram_tensor("hidden_t", [d, T], F32R, kind="ExternalInput")
    w_qk_t = nc.dram_tensor("w_qk_t", [d, 2 * QBLK], F32R, kind="ExternalInput")
    w_v_t = nc.dram_tensor("w_v_t", [d, QBLK], F32R, kind="ExternalInput")
    wo2 = nc.dram_tensor("wo2", [DC, P, ECH * P], BF16, kind="ExternalInput")
    cos2 = nc.dram_tensor("cos2", [P, s], F32, kind="ExternalInput")
    sinrot2 = nc.dram_tensor("sinrot2", [P, s], F32, kind="ExternalInput")
    rotm_d = nc.dram_tensor("rotm", [P, P], F32R, kind="ExternalInput")
    ones_d = nc.dram_tensor("onesc", [P, 64], F32R, kind="ExternalInput")
    out_t = nc.dram_tensor("out_t", [d, TS], F32, kind="ExternalOutput")
    if debug:
        dbg_qk = nc.dram_tensor("dbg_qk", [P, 6, s], F32, kind="ExternalOutput")
        dbg_v = nc.dram_tensor("dbg_v", [P, s // P, EVA], BF16, kind="ExternalOutput")
        dbg_ao = nc.dram_tensor("dbg_ao", [b, hd, h_loc, s], BF16, kind="ExternalOutput")
        dbg_asl = nc.dram_tensor("dbg_asl", [P, ECH, TS], BF16, kind="ExternalOutput")
        dbg_sm = nc.dram_tensor("dbg_sm", [2 + 64, 512], F32, kind="ExternalOutput")

    hid_v = hidden_t[:].rearrange("(c p) t -> p c t", p=P)
    wqk_v = w_qk_t[:].rearrange("(c p) e -> p c e", p=P)
    wv_v = w_v_t[:].rearrange("(c p) e -> p c e", p=P)

    with tile.TileContext(nc) as tc:
        with (
            tc.tile_pool(name="dramp", bufs=1, space="DRAM") as dramp,
            tc.tile_pool(name="tabs", bufs=1) as tabs,
            tc.tile_pool(name="qkp", bufs=2) as qkp,
            tc.tile_pool(name="vp", bufs=2) as vp,
            tc.tile_pool(name="ep", bufs=3) as ep,
            tc.tile_pool(name="ropep", bufs=2) as ropep,
            tc.tile_pool(name="dnp", bufs=2) as dnp,
            tc.tile_pool(name="dnrp", bufs=2) as dnrp,
            tc.tile_pool(name="aop", bufs=1) as aop,
            tc.tile_pool(name="psQ", bufs=3, space="PSUM") as psQ,
            tc.tile_pool(name="psS", bufs=3, space="PSUM") as psS,
            tc.tile_pool(name="psO", bufs=2, space="PSUM") as psO,
        ):
            cc_in_h = [dramp.tile([n_cores, 2 * P, 2 * SH], BF16, name=f"cc_in_{q}")
                       for q in range(b // 2)]
            cc_out_h = [dramp.tile([n_cores, 2 * P, 2 * SH], BF16, name=f"cc_out_{q}")
                        for q in range(b // 2)]

            cos_sb = tabs.tile([P, s], F32)
            sin_sb = tabs.tile([P, s], F32)
            rotm_sb = tabs.tile([P, P], F32R)
            ones_sb = tabs.tile([P, 64], F32R)
            nc.sync.dma_start(cos_sb[:], cos2[:])
            nc.sync.dma_start(sin_sb[:], sinrot2[:])
            nc.sync.dma_start(rotm_sb[:], rotm_d[:])
            nc.sync.dma_start(ones_sb[:], ones_d[:])

            qk_tiles = {}
            v_tiles = {}
            ao_tiles = {}

            def qkv_units(bi):
                """Generator: emit QKV proj + RoPE for batch bi in small PE
                units so the driver can weave them between SDPA steps."""
                qk_tiles[bi] = qkp.tile([P, 6, s], F32R, tag="qk", name="qk_t")
                v_tiles[bi] = vp.tile([P, KTC, EVA], BF16, tag="v", name="v_t")
                for h in range(h_loc):
                    nc.scalar.activation(
                        v_tiles[bi][:, :, h * (hd + 1) + hd:h * (hd + 1) + hd + 1],
                        wv_sb[:, 0:KTC, 0:1],
                        mybir.ActivationFunctionType.Identity,
                        bias=1.0, scale=0.0)
                qk_t = qk_tiles[bi]
                v_t = v_tiles[bi]
                # zero the unused half of each per-head q chunk so the scores
                # matmul can run full-128-contraction (stale lhsT rows hit 0s)
                for h in range(h_loc):
                    z0 = (1 - h % 2) * 64
                    nc.gpsimd.memset(
                        qk_t[z0:z0 + 64, h:h + 1, :].bitcast(F32), 0.0)
                yield
                for tci in range(NTC):
                    s0 = tci * TCH
                    t0 = bi * s + s0
                    hid_sb = hidp.tile([P, DCH, TCH], F32R, tag="hid")
                    nc.sync.dma_start(hid_sb[:], hid_v[:, :, t0:t0 + TCH])
                    yield
                    for tsub in range(TCH // P):
                        kc = tci * (TCH // P) + tsub
                        psv = psQ.tile([P, QT], F32, tag="psq", name="psv")[:, 0:QBLK]
                        for dd in range(DCH):
                            nc.tensor.matmul(
                                psv[:], lhsT=hid_sb[:, dd, tsub * P:(tsub + 1) * P],
                                rhs=wv_sb[:, dd, :],
                                start=(dd == 0), stop=(dd == DCH - 1),
                            )
                            if dd % 4 == 3:
                                yield
                        nc.vector.tensor_copy(
                            v_t[:, kc:kc + 1, :].rearrange(
                                "p o (h e) -> p (o h) e", h=h_loc)[:, :, 0:hd],
                            psv.rearrange("p (h e) -> p h e", e=hd),
                        )
                    for ec in range(4):
                        ps = psQ.tile([P, QT], F32, tag="psq", name="psqk")[:, 0:TCH]
                        for dd in range(DCH):
                            nc.tensor.matmul(
                                ps[:], lhsT=wqk_sb[:, dd, ec * P:(ec + 1) * P],
                                rhs=hid_sb[:, dd, :],
                                start=(dd == 0), stop=(dd == DCH - 1),
                            )
                            if dd % 4 == 3:
                                yield
                        raw = ropep.tile([P, TCH], F32R, tag="raw")
                        nc.vector.tensor_copy(raw[:], ps[:])
                        psr = psQ.tile([P, QT], F32, tag="psq", name="psr")[:, 0:TCH]
                        nc.tensor.matmul(psr[:], lhsT=rotm_sb[:], rhs=raw[:],
                                         start=True, stop=True)
                        cp = ropep.tile([P, TCH], F32, tag="cp")
                        nc.vector.tensor_tensor(cp[:], ps[:], cos_sb[:, s0:s0 + TCH], MULT)
                        sw = ropep.tile([P, TCH], F32, tag="sw")
                        nc.vector.tensor_tensor(sw[:], psr[:], sin_sb[:, s0:s0 + TCH], MULT)
                        if ec < 2:
                            # q: head 2*ec in rows 0:64, head 2*ec+1 in 64:128
                            nc.vector.tensor_tensor(
                                qk_t[0:64, 2 * ec, s0:s0 + TCH],
                                cp[0:64, :], sw[0:64, :], ADD)
                            nc.vector.tensor_tensor(
                                qk_t[64:128, 2 * ec + 1, s0:s0 + TCH],
                                cp[64:128, :], sw[64:128, :], ADD)
                        else:
                            nc.vector.tensor_tensor(
                                qk_t[:, 2 + ec, s0:s0 + TCH], cp[:], sw[:], ADD)
                        yield

            def pull(gen, n):
                if gen is None:
                    return
                for _ in range(n):
                    try:
                        next(gen)
                    except StopIteration:
                        return

            def make_normalize(bi, h, qt, pso):
                """Deferred normalize: emitted a few SDPA steps later so the
                PE queue never head-of-line blocks on the ACT Ln/Exp chain."""
                ao_all = ao_tiles[bi]
                q0 = qt * QT
                # 1/denom = exp(-ln(denom)) on ACT (Ln and Exp share a table)
                dnln = dnp.tile([hd + 1, QT], F32, tag="dnln", name="dnln")
                nc.scalar.activation(dnln[hd:hd + 1, :], pso[hd:hd + 1, :], LN)
                dninv = dnp.tile([hd + 1, QT], F32R, tag="dninv", name="dninv")
                nc.scalar.activation(dninv[hd:hd + 1, :], dnln[hd:hd + 1, :],
                                     EXP, scale=-1.0)

                def fin():
                    rep = psS.tile([P, QT], F32, tag="pss", name="rep")[0:hd, :]
                    nc.tensor.matmul(rep[:], lhsT=ones_sb[hd:hd + 1, 0:hd],
                                     rhs=dninv[hd:hd + 1, :], start=True, stop=True)
                    dnrep = dnrp.tile([hd, QT], F32, tag="dnrep", name="dnrep")
                    nc.vector.tensor_copy(dnrep[:], rep[:])
                    nc.vector.tensor_tensor(
                        ao_all[:, h, q0:q0 + QT], pso[0:hd, :], dnrep[:], MULT)
                return fin

            def emit_sdpa_batch(bi, gen, kt_pull_mod=1):
                """All 8 SDPA groups of batch bi, weaving in the next batch's
                QKV units (gen) to keep the PE dense and warm."""
                ao_tiles[bi] = aop.tile([hd, h_loc, s], BF16, tag="ao", name="ao_all")
                qk_t = qk_tiles[bi]
                v_t = v_tiles[bi]
                pending = None
                for h in range(h_loc):
                    for qt in range(NQT):
                        kchunk = 4 + h // 2
                        q0 = qt * QT
                        pso = psO.tile([hd + 1, QT], F32, tag="pso", name="pso")
                        prev_e = None
                        for kt in range(KTC):
                            ss = psS.tile([P, QT], F32, tag="pss", name="ss")
                            nc.tensor.matmul(
                                ss[:],
                                lhsT=qk_t[:, kchunk, kt * P:(kt + 1) * P],
                                rhs=qk_t[:, h, q0:q0 + QT],
                                start=True, stop=True,
                            )
                            e_t = ep.tile([P, QT], BF16, tag="e", name="e_t")
                            nc.scalar.activation(e_t[:], ss[:], EXP)
                            if prev_e is not None:
                                nc.tensor.matmul(
                                    pso[:],
                                    lhsT=v_t[:, kt - 1, h * (hd + 1):(h + 1) * (hd + 1)],
                                    rhs=prev_e[:],
                                    start=(kt == 1), stop=False,
                                )
                            prev_e = e_t
                            if kt % kt_pull_mod == 0:
                                pull(gen, 1)
                            if kt == 2 and pending is not None:
                                pending()
                                pending = None
                        nc.tensor.matmul(
                            pso[:],
                            lhsT=v_t[:, KTC - 1, h * (hd + 1):(h + 1) * (hd + 1)],
                            rhs=prev_e[:],
                            start=False, stop=True,
                        )
                        pending = make_normalize(bi, h, qt, pso)
                        pull(gen, 2)
                pull(gen, 4)
                pending()
                if kt_pull_mod == 1:
                    pull(gen, 1000)  # drain leftovers before staging
                # stage this batch's attention outputs for the AllToAll
                ao_all = ao_tiles[bi]
                h2 = bi // 2
                for jj in range(4):
                    j = (bi % 2) * 4 + jj
                    dst = cc_in_h[h2][j].rearrange("(h p) t -> p h t", p=hd)
                    nc.sync.dma_start(dst, ao_all[:, :, jj * 2 * SH:(jj + 1) * 2 * SH])
                if debug:
                    nc.sync.dma_start(dbg_ao[bi], ao_all[:])
                    if bi == 0:
                        nc.sync.dma_start(dbg_qk[:], qk_tiles[0][:].bitcast(F32))
                        nc.sync.dma_start(dbg_v[:], v_tiles[0][:])
                if bi % 2 == 1:
                    nc.gpsimd.collective_compute(
                        "AllToAll",
                        mybir.AluOpType.bypass,
                        replica_groups=[list(range(n_cores))],
                        ins=[cc_in_h[h2].opt()],
                        outs=[cc_out_h[h2].opt()],
                    )
                pull(gen, 1000)  # leftover PE work fills the collective wait

            with (
                tc.tile_pool(name="wqkp", bufs=1) as wqkp,
                tc.tile_pool(name="hidp", bufs=2) as hidp,
            ):
                wqk_sb = wqkp.tile([P, DCH, 2 * QBLK], F32R)
                wv_sb = wqkp.tile([P, DCH, QBLK], F32R)
                for dd4 in range(0, DCH, DCH // 2):
                    nc.sync.dma_start(wv_sb[:, dd4:dd4 + DCH // 2],
                                      wv_v[:, dd4:dd4 + DCH // 2])
                for dd4 in range(0, DCH, DCH // 4):
                    nc.sync.dma_start(wqk_sb[:, dd4:dd4 + DCH // 4],
                                      wqk_v[:, dd4:dd4 + DCH // 4])

                g0 = qkv_units(0)
                pull(g0, 1000)
                for bi in range(b - 1):
                    emit_sdpa_batch(bi, qkv_units(bi + 1))

            # wqk/wv/hid SBUF released here -> o_proj pools alias that space
            with (
                tc.tile_pool(name="aslp", bufs=1) as aslp,
                tc.tile_pool(name="wop", bufs=16) as wop,
                tc.tile_pool(name="outp", bufs=3) as outp,
            ):
                asl = aslp.tile([P, ECH, TS], BF16)
                cc_v = cc_out_h[0][:].rearrange("j (ci p) t -> p (j ci) t", p=P)
                nc.sync.dma_start(asl[:, :, 0:2 * SH], cc_v)

                def oproj_units(h2):
                    """Transposed o_proj for token half h2:
                    out_t[:, h2*256:(h2+1)*256] = w_o @ asl[:, :, half]."""
                    c0 = h2 * 2 * SH
                    for dc in range(DC):
                        wo_sb = wop.tile([P, ECH * P], BF16, tag="wo", name="wo_sb")
                        nc.sync.dma_start(wo_sb[:], wo2[dc])
                        yield
                        pso = psQ.tile([P, QT], F32, tag="psq",
                                       name="psop")[:, 0:2 * SH]
                        for e in range(ECH):
                            nc.tensor.matmul(
                                pso[:], lhsT=wo_sb[:, e * P:(e + 1) * P],
                                rhs=asl[:, e, c0:c0 + 2 * SH],
                                start=(e == 0), stop=(e == ECH - 1),
                            )
                            if e % 8 == 7:
                                yield
                        ob = outp.tile([P, 2 * SH], F32, tag="ob", name="ob")
                        nc.vector.tensor_copy(ob[:], pso[:])
                        nc.scalar.dma_start(
                            out_t[dc * P:(dc + 1) * P, c0:c0 + 2 * SH], ob[:])
                        yield

                emit_sdpa_batch(b - 1, oproj_units(0), kt_pull_mod=4)
                cc_v = cc_out_h[1][:].rearrange("j (ci p) t -> p (j ci) t", p=P)
                nc.sync.dma_start(asl[:, :, 2 * SH:4 * SH], cc_v)

                if debug:
                    nc.sync.dma_start(dbg_asl[:], asl[:])
                for _ in oproj_units(1):
                    pass
    nc.finalize()
    return nc


def prep_inputs(cos, sin, hidden_states, w_qkv, w_o,
                b=B, s=S, d=D, h_loc=H_LOC, hd=HD, n_cores=N_CORES):
    """Host-side sharding/layout: returns per-core input maps."""
    cos = np.asarray(cos, dtype=np.float32)
    sin = np.asarray(sin, dtype=np.float32)
    hidden_states = np.asarray(hidden_states, dtype=np.float32)
    w_qkv = np.asarray(w_qkv, dtype=np.float32)
    w_o = np.asarray(w_o, dtype=np.float32)

    T = b * s
    P = 128
    QBLK = h_loc * hd
    HHD = n_cores * QBLK  # total H*HD

    hidden_t = np.ascontiguousarray(hidden_states.reshape(T, d).T)

    # wo2[dc, p, ec*128+j] = w_o[dc*128+j, ec*128+p]
    import ml_dtypes
    wo2 = np.ascontiguousarray(
        w_o.reshape(d // P, P, HHD // P, P).transpose(0, 3, 2, 1)
        .reshape(d // P, P, HHD)).astype(ml_dtypes.bfloat16)

    cos_t = cos.T  # [hd, s]
    sin_t = sin.T
    cos2 = np.ascontiguousarray(np.tile(cos_t, (P // hd, 1)))
    srt = sin_t.copy()
    srt[0:hd // 2] = -sin_t[0:hd // 2]
    sinrot2 = np.ascontiguousarray(np.tile(srt, (P // hd, 1)))

    onesc = np.ones((P, 64), dtype=np.float32)

    # rotate-half permutation (sign lives in sinrot2)
    rotm = np.zeros((P, P), dtype=np.float32)
    for m in range(P):
        pp = m + hd // 2 if (m % hd) < hd // 2 else m - hd // 2
        rotm[pp, m] = 1.0

    maps = []
    for c in range(n_cores):
        wq = w_qkv[c * QBLK:(c + 1) * QBLK] * 0.125
        wk = w_qkv[HHD + c * QBLK:HHD + (c + 1) * QBLK]
        wv = w_qkv[2 * HHD + c * QBLK:2 * HHD + (c + 1) * QBLK]
        w_qk_t = np.ascontiguousarray(np.concatenate([wq, wk], axis=0).T)
        w_v_t = np.ascontiguousarray(wv.T)
        maps.append({
            "hidden_t": hidden_t,
            "w_qk_t": w_qk_t,
            "w_v_t": w_v_t,
            "wo2": wo2,
            "cos2": cos2,
            "sinrot2": sinrot2,
            "rotm": rotm,
            "onesc": onesc,
        })
    return maps


_NC_CACHE = {}


def run(inputs, trace=False, dims=None, debug=False):
    """Run the distributed kernel. Returns (full_output, BassKernelResults)."""
    dims = dims or dict(b=B, s=S, d=D, h_loc=H_LOC, hd=HD, n_cores=N_CORES)
    key = tuple(sorted(dims.items())) + (debug,)
    if key not in _NC_CACHE:
        _NC_CACHE[key] = build_attention(debug=debug, **dims)
    nc = _NC_CACHE[key]
    maps = prep_inputs(inputs["cos"], inputs["sin"], inputs["hidden_states"],
                       inputs["w_qkv"], inputs["w_o"], **dims)
    res = run_bass_kernel_spmd(nc, maps, list(range(dims["n_cores"])), trace=trace)
    n_cores = dims["n_cores"]
    b, s, d = dims["b"], dims["s"], dims["d"]
    SH2 = 2 * s // n_cores                       # 256 tokens per core per half
    out = np.empty((2, n_cores, SH2, d), dtype=np.float32)
    for c in range(n_cores):
        ot = res.results[c]["out_t"]            # [d, 2*SH2]
        out[:, c] = ot.reshape(d, 2, SH2).transpose(1, 2, 0)
    return out.reshape(b, s, d), res


def kernel(**inputs) -> np.ndarray:
    out, _ = run(inputs)
    return out
